# revision 1
# baseline (speedup 1.0000x reference)
"""BitLinear (ternary weight) inference kernel for Trainium2, 8-core SPMD.

Full-input contract: kernel(**inputs) takes the complete tensors and returns
the complete output. The batch dim (B=8) is sharded 1:1 onto the 8
NeuronCores; each core computes y[b] = x[b] @ (w_q * 2^s_exp)^T + bias as a
2048^3 matmul (fp16 x, fp8 w, fp32 PSUM accumulation).

Host prep (cheap, O(bytes)): fold the power-of-two per-channel scale into
the ternary weights — values +-2^s / 0 are EXACT in fp8e4m3 — transpose
both operands into the PE's contraction-major [K, ...] layout, cast x to
fp16 (the only lossy step, ~2^-11 relative), broadcast bias to [128, OUT].

Device schedule (PE-bound; ~245us/core vs 218.5us matmul streaming floor):
  - Mixed-dtype matmuls: stationary x-tile fp16 [128,128], moving w fp8
    [128,512], one PSUM bank each, K accumulated 128 rows per step.
  - The first 6 row tiles run k-chunks 0..3 as soon as ~2 MiB of input has
    landed (pass A), parking partial sums in SBUF; the remaining k-chunks
    are added later (accum pass) interleaved with full-k single-pass tiles,
    so the PE never waits on the 12.6 MiB input stream.
  - Inputs on the Sync HWDGE ring, output stores on the Scalar HWDGE ring,
    epilogue (psum + bias / + partial -> SBUF) on the Vector engine, and a
    short dummy-matmul burst pre-warms the PE HAM clock gate.
"""
import os

import ml_dtypes
import numpy as np

B, T, IN, OUT = 8, 2048, 2048, 2048
P = 128
NCORES = 8
NF = 512        # matmul free dim (one PSUM bank of fp32)
KA = 4          # k-chunks in pass A (first-pass dependency set = KA MiB won't gate PE)

last_exec_time_ns = None
_CACHE = {}


def _install_prof_shim():
    """Make antenv.axon_hooks importable so trace=True works under axon."""
    import sys
    import types

    if "antenv.axon_hooks" in sys.modules:
        return
    try:
        from trn_agent_boot.trn_boot import _ntff_profile_via_ctypes
    except ImportError:
        return
    hook = _ntff_profile_via_ctypes("/opt/axon/libaxon_pjrt.so")
    mod = types.ModuleType("antenv.axon_hooks")
    mod.get_axon_ntff_profile_hook = lambda: hook
    mod.set_axon_ntff_profile_hook = lambda h: None
    sys.modules["antenv.axon_hooks"] = mod


def _build():
    import concourse.bacc as bacc
    import concourse.mybir as mybir
    from concourse.tile import TileContext

    nc = bacc.Bacc()
    x = nc.dram_tensor("x", (IN, T), mybir.dt.float16, kind="ExternalInput")
    w = nc.dram_tensor("w", (IN, OUT), mybir.dt.float8e4, kind="ExternalInput")
    bias = nc.dram_tensor("bias", (P, OUT), mybir.dt.float32, kind="ExternalInput")
    y = nc.dram_tensor("y", (T, OUT), mybir.dt.float32, kind="ExternalOutput")

    KT = IN // P    # contraction chunks
    TT = T // P     # output row tiles
    OC = OUT // NF  # psum banks per row tile

    HOUT = OUT // 2  # two psum tiles (2 banks each) per row tile

    with TileContext(nc) as tc:
        with tc.tile_pool(name="wp", bufs=1) as wp, \
             tc.tile_pool(name="xp", bufs=1) as xp, \
             tc.tile_pool(name="bp", bufs=1) as bp, \
             tc.tile_pool(name="op", bufs=4) as op_, \
             tc.tile_pool(name="ptp", bufs=1) as ptp, \
             tc.tile_pool(name="pp", bufs=4, space="PSUM") as pp:

            # Interleave w/x chunk loads k-wise so pass A's working set
            # (k < KA) lands first and the PE can start after ~2 MiB.
            # Later chunks load pairwise (>=1 MiB DMAs for efficiency).
            w_tiles = [None] * KT
            xT_tiles = [None] * KT
            bias_t = bp.tile([P, OUT], mybir.dt.float32, tag="bias")
            x3 = x.rearrange("(ko p) t -> p ko t", p=P)
            w3 = w.rearrange("(ko p) o -> p ko o", p=P)

            # HAM pre-warm: a short burst of dummy matmuls on a scratch tile
            # while the first loads are in flight, so the PE clock-gate is
            # near 8/8 when the real matmuls start. Uses one "ps" slot
            # briefly (released well before pass A needs its 4th buffer).
            warm_sb = bp.tile([P, NF], mybir.dt.float16, tag="warm")
            nc.gpsimd.memset(warm_sb, 0.0)
            warm_ps = pp.tile([P, HOUT], mybir.dt.float32, tag="ps",
                              name="warmps")
            for i in range(6):
                nc.tensor.matmul(warm_ps[:, :NF], warm_sb[:, :P], warm_sb,
                                 start=(i == 0), stop=(i == 5))

            HT = T // 2
            for k in range(KA):
                wt = wp.tile([P, OUT], mybir.dt.float8e4, tag=f"w{k}")
                xt = xp.tile([P, T], mybir.dt.float16, tag=f"x{k}")
                nc.sync.dma_start(wt, w[k * P:(k + 1) * P, :])
                nc.sync.dma_start(xt[:, :HT], x[k * P:(k + 1) * P, :HT])
                w_tiles[k] = wt
                xT_tiles[k] = xt
            nc.sync.dma_start(bias_t, bias[:, :])
            for k in range(KA, KT, 2):
                wt2 = wp.tile([P, 2, OUT], mybir.dt.float8e4, tag=f"w{k}")
                nc.sync.dma_start(wt2, w3[:, k:k + 2, :])
                w_tiles[k] = wt2[:, 0]
                w_tiles[k + 1] = wt2[:, 1]
                xt2 = xp.tile([P, 2, T], mybir.dt.float16, tag=f"x{k}")
                nc.sync.dma_start(xt2, x3[:, k:k + 2, :])
                xT_tiles[k] = xt2[:, 0]
                xT_tiles[k + 1] = xt2[:, 1]
            # deferred: t>=1024 halves of the pass-A x chunks are only read
            # by single-pass row tiles 8+, which run ~50us after this lands
            for k in range(KA):
                nc.sync.dma_start(xT_tiles[k][:, HT:], x[k * P:(k + 1) * P, HT:])

            TSPLIT = 6       # row tiles 0..TSPLIT-1 two-pass (partials in SBUF)

            partial_tiles = [
                ptp.tile([P, OUT], mybir.dt.float32, tag=f"pt{j}", name=f"pt{j}")
                for j in range(TSPLIT)
            ]

            def do_tiles(tt_range, k_lo, k_hi, mode):
                # mode: "partial" = bias add into SBUF partial (no store),
                #       "accum" = add SBUF partial + store,
                #       "single" = bias add + store
                for tt in tt_range:
                    pss = [pp.tile([P, HOUT], mybir.dt.float32, tag="ps",
                                   name=f"ps{h}") for h in range(2)]
                    for k in range(k_lo, k_hi):
                        lhsT = xT_tiles[k][:, tt * P:(tt + 1) * P]
                        for oc in range(OC):
                            ps = pss[oc // 2]
                            lo = (oc % 2) * NF
                            nc.tensor.matmul(
                                ps[:, lo:lo + NF],
                                lhsT,
                                w_tiles[k][:, oc * NF:(oc + 1) * NF],
                                start=(k == k_lo),
                                stop=(k == k_hi - 1),
                            )
                    if mode == "partial":
                        ot = partial_tiles[tt]
                    else:
                        ot = op_.tile([P, OUT], mybir.dt.float32, tag="out")
                    if tt == TT - 1:
                        # last tile: chunk epilogue+store so the store of
                        # chunk q overlaps the add of chunk q+1 (short tail)
                        for q in range(OC):
                            sl = slice(q * NF, (q + 1) * NF)
                            psl = slice((q % 2) * NF, (q % 2) * NF + NF)
                            nc.vector.tensor_add(ot[:, sl], pss[q // 2][:, psl],
                                                 bias_t[:, sl])
                            eng = nc.scalar if q % 2 == 0 else nc.sync
                            eng.dma_start(y[tt * P:(tt + 1) * P, sl],
                                          ot[:, sl])
                        continue
                    for h in range(2):
                        sl = slice(h * HOUT, (h + 1) * HOUT)
                        if mode == "accum":
                            nc.vector.tensor_add(ot[:, sl], pss[h],
                                                 partial_tiles[tt][:, sl])
                        else:
                            nc.vector.tensor_add(ot[:, sl], pss[h], bias_t[:, sl])
                    if mode != "partial":
                        nc.scalar.dma_start(y[tt * P:(tt + 1) * P, :], ot)

            do_tiles(range(TSPLIT), 0, KA, "partial")
            # Interleave accum and single-pass tiles so the PE always has
            # runnable chunks while the tail of the input load streams in.
            for j in range(TT - TSPLIT):
                if j < TSPLIT:
                    do_tiles([j], KA, KT, "accum")
                do_tiles([TSPLIT + j], 0, KT, "single")

    nc.compile()
    return nc


def kernel(x, w_q, s_exp, bias):
    global last_exec_time_ns
    from concourse.bass_utils import run_bass_kernel_spmd

    x = np.asarray(x)
    w_q = np.asarray(w_q)
    s_exp = np.asarray(s_exp)
    bias = np.asarray(bias, dtype=np.float32)
    assert x.shape == (B, T, IN) and w_q.shape == (OUT, IN)

    # Fold the power-of-two per-output-channel scale into the ternary
    # weights: values are +-2^s or 0 with s in [-8, 0], exact in fp8e4m3
    # (2^-8 and 2^-9 are exact subnormals).
    scale = np.exp2(s_exp.astype(np.float32))
    w_scaled_t = (w_q.astype(np.float32) * scale[:, None]).T
    w_fp8 = np.ascontiguousarray(w_scaled_t).astype(ml_dtypes.float8_e4m3fn)
    if not np.array_equal(w_fp8.astype(np.float32), w_scaled_t):
        import warnings
        warnings.warn("scaled ternary weights not exact in fp8e4m3; "
                      "proceeding with rounded weights")
    bias_bcast = np.ascontiguousarray(
        np.broadcast_to(bias.astype(np.float32), (P, OUT)))
    # Contraction-major layout for the PE: x^T[b] = [IN, T], fp16.
    xT_f16 = np.ascontiguousarray(
        x.astype(np.float16).transpose(0, 2, 1))

    nc = _CACHE.get("nc")
    if nc is None:
        nc = _CACHE["nc"] = _build()

    in_maps = [
        {"x": xT_f16[b], "w": w_fp8, "bias": bias_bcast} for b in range(B)
    ]

    trace = bool(int(os.environ.get("BITLIN_TRACE", "0")))
    if trace:
        _install_prof_shim()
    res = run_bass_kernel_spmd(nc, in_maps, list(range(NCORES)), trace=trace)
    last_exec_time_ns = res.exec_time_ns

    out = np.stack([res.results[b]["y"] for b in range(B)], axis=0)
    return out.astype(np.float32, copy=False)



# revision 2
# speedup vs baseline: 1.3569x; 1.3569x over previous
"""BitLinear (ternary weight) inference kernel for Trainium2, 8-core SPMD.

Full-input contract: kernel(**inputs) takes the complete tensors and returns
the complete output. The batch dim (B=8) is sharded 1:1 onto the 8
NeuronCores; each core computes y[b] = x[b] @ (w_q * 2^s_exp)^T + bias as a
2048^3 matmul in fp8 with DoubleRow perf mode (2 fp8 MACs/cell/cycle).

Numerics: pure-fp8 x quantization gives rel err 0.024 (> the 2e-2 gate), so
the kernel adds a correction matmul on the NCORR=512 output channels with the
largest 2^s scale (they dominate the error): host permutes the channels so
those are columns 0:512, and e = 64*(x - fp8(x)) quantized to fp8 against
wc = w * 2^-6 accumulates the correction into the same PSUM bank as the main
matmul (64 * 2^-6 = 1). Measured rel err 5.8e-3. All weight values are exact
in fp8e4m3 (+-2^s, s in [-8,0]; wc needs s >= -3, satisfied by construction).

Device schedule (PE-bound; ~137-155us/core fp8 DoubleRow floor vs 218.5us
fp16): per (row tile, k-pair) 4 main matmuls [128,2,128]x[128,2,512] plus 1
correction matmul, PSUM fp32. Two-pass tiles 0..5 with arrival-matched pass-A
k-extents park partials in SBUF so the PE never waits on the 14.25 MiB input
stream (sync ring, ~400 GB/s measured); stores ride the scalar ring.
"""
import os

import ml_dtypes
import numpy as np

B, T, IN, OUT = 8, 2048, 2048, 2048
P = 128
NCORES = 8
NF = 512        # matmul free dim (one PSUM bank of fp32)
KP = 8          # k-pairs (K=256 per DoubleRow matmul)
NCORR = 512     # corrected output channels (permuted to the front)
TSPLIT = 6
KA = [2, 2, 3, 4, 5, 6]   # pass-A k-pair extent per two-pass tile

last_exec_time_ns = None
_CACHE = {}


def _install_prof_shim():
    """Make antenv.axon_hooks importable so trace=True works under axon."""
    import sys
    import types

    if "antenv.axon_hooks" in sys.modules:
        return
    try:
        from trn_agent_boot.trn_boot import _ntff_profile_via_ctypes
    except ImportError:
        return
    hook = _ntff_profile_via_ctypes("/opt/axon/libaxon_pjrt.so")
    mod = types.ModuleType("antenv.axon_hooks")
    mod.get_axon_ntff_profile_hook = lambda: hook
    mod.set_axon_ntff_profile_hook = lambda h: None
    sys.modules["antenv.axon_hooks"] = mod


def _build():
    import concourse.bacc as bacc
    import concourse.mybir as mybir
    from concourse.tile import TileContext

    DR = mybir.MatmulPerfMode.DoubleRow

    nc = bacc.Bacc()
    x8 = nc.dram_tensor("x8", (IN, T), mybir.dt.float8e4, kind="ExternalInput")
    e8 = nc.dram_tensor("e8", (IN, T), mybir.dt.float8e4, kind="ExternalInput")
    w = nc.dram_tensor("w", (IN, OUT), mybir.dt.float8e4, kind="ExternalInput")
    wc = nc.dram_tensor("wc", (IN, NCORR), mybir.dt.float8e4,
                        kind="ExternalInput")
    bias = nc.dram_tensor("bias", (P, OUT), mybir.dt.float32, kind="ExternalInput")
    y = nc.dram_tensor("y", (T, OUT), mybir.dt.float32, kind="ExternalOutput")

    TT = T // P     # output row tiles
    OC = OUT // NF  # psum banks per row tile
    HT = T // 2

    # DoubleRow k-pair layout: [p, pair, free] where row = pair*... is
    # (2m+i)*128 + p for tile m slice [:, 2m:2m+2, :].
    x3 = x8.rearrange("(ko p) t -> p ko t", p=P)
    e3 = e8.rearrange("(ko p) t -> p ko t", p=P)
    w3 = w.rearrange("(ko p) o -> p ko o", p=P)
    wc3 = wc.rearrange("(ko p) o -> p ko o", p=P)

    with TileContext(nc) as tc:
        with tc.tile_pool(name="wp", bufs=1) as wp, \
             tc.tile_pool(name="xp", bufs=1) as xp, \
             tc.tile_pool(name="ep", bufs=1) as ep, \
             tc.tile_pool(name="wcp", bufs=1) as wcp, \
             tc.tile_pool(name="bp", bufs=1) as bp, \
             tc.tile_pool(name="op", bufs=4) as op_, \
             tc.tile_pool(name="ptp", bufs=1) as ptp, \
             tc.tile_pool(name="pp", bufs=2, space="PSUM") as pp:

            bias_t = bp.tile([P, OUT], mybir.dt.float32, tag="bias")

            # HAM pre-warm: dummy-matmul burst (~3.8us cold) while the first
            # loads are in flight, so the PE clock gate is 8/8 when the real
            # matmuls start.
            warm_sb = bp.tile([P, NF], mybir.dt.float16, tag="warm")
            nc.gpsimd.memset(warm_sb, 0.0)
            warm_ps = pp.tile([P, OUT], mybir.dt.float32, tag="ps",
                              name="warmps")
            for i in range(9):
                nc.tensor.matmul(warm_ps[:, :NF], warm_sb[:, :P], warm_sb,
                                 start=(i == 0), stop=(i == 8))

            w_tiles = [None] * KP
            wc_tiles = [None] * KP
            x_tiles = [None] * KP
            e_tiles = [None] * KP

            def load_pair(m, t_hi):
                """Queue loads for k-pair m (x/e up to t_hi columns)."""
                wt = wp.tile([P, 2, OUT], mybir.dt.float8e4, tag=f"w{m}")
                nc.sync.dma_start(wt, w3[:, 2 * m:2 * m + 2, :])
                w_tiles[m] = wt
                wct = wcp.tile([P, 2, NCORR], mybir.dt.float8e4, tag=f"wc{m}")
                nc.sync.dma_start(wct, wc3[:, 2 * m:2 * m + 2, :])
                wc_tiles[m] = wct
                xt = xp.tile([P, 2, T], mybir.dt.float8e4, tag=f"x{m}")
                nc.sync.dma_start(xt[:, :, :t_hi], x3[:, 2 * m:2 * m + 2, :t_hi])
                x_tiles[m] = xt
                et = ep.tile([P, 2, T], mybir.dt.float8e4, tag=f"e{m}")
                nc.sync.dma_start(et[:, :, :t_hi], e3[:, 2 * m:2 * m + 2, :t_hi])
                e_tiles[m] = et

            # Pass-A pairs first with only the t<1024 halves of x/e (pass-A
            # tiles 0..5 read t<768); the rest streams in behind.
            load_pair(0, HT)
            load_pair(1, HT)
            nc.sync.dma_start(bias_t, bias[:, :])
            for m in range(2, KP):
                load_pair(m, T)
            for m in range(2):
                nc.sync.dma_start(x_tiles[m][:, :, HT:],
                                  x3[:, 2 * m:2 * m + 2, HT:])
                nc.sync.dma_start(e_tiles[m][:, :, HT:],
                                  e3[:, 2 * m:2 * m + 2, HT:])

            partial_tiles = [
                ptp.tile([P, OUT], mybir.dt.float32, tag=f"pt{j}", name=f"pt{j}")
                for j in range(TSPLIT)
            ]

            def do_tile(tt, p_lo, p_hi, mode):
                # mode: "partial" = bias add into SBUF partial (no store),
                #       "accum" = add SBUF partial + store,
                #       "single" = bias add + store
                ps = pp.tile([P, OUT], mybir.dt.float32, tag="ps",
                             name=f"ps{tt}.{p_lo}")
                for pr in range(p_lo, p_hi):
                    lhsT = x_tiles[pr][:, :, tt * P:(tt + 1) * P]
                    for oc in range(OC):
                        nc.tensor.matmul(
                            ps[:, oc * NF:(oc + 1) * NF],
                            lhsT,
                            w_tiles[pr][:, :, oc * NF:(oc + 1) * NF],
                            start=(pr == p_lo),
                            stop=(pr == p_hi - 1 and oc > 0),
                            perf_mode=DR,
                        )
                    # correction accumulates into bank 0 (channels 0:512)
                    nc.tensor.matmul(
                        ps[:, 0:NF],
                        e_tiles[pr][:, :, tt * P:(tt + 1) * P],
                        wc_tiles[pr],
                        start=False,
                        stop=(pr == p_hi - 1),
                        perf_mode=DR,
                    )
                if mode == "partial":
                    ot = partial_tiles[tt]
                else:
                    ot = op_.tile([P, OUT], mybir.dt.float32, tag="out")
                other = partial_tiles[tt] if mode == "accum" else bias_t
                if tt == TT - 1:
                    # last tile: chunk epilogue+store so stores overlap the
                    # remaining adds and split across both DMA rings
                    for q in range(OC):
                        sl = slice(q * NF, (q + 1) * NF)
                        nc.vector.tensor_add(ot[:, sl], ps[:, sl], other[:, sl])
                        eng = nc.scalar if q % 2 == 0 else nc.sync
                        eng.dma_start(y[tt * P:(tt + 1) * P, sl], ot[:, sl])
                    return
                for h in range(2):
                    sl = slice(h * OUT // 2, (h + 1) * OUT // 2)
                    nc.vector.tensor_add(ot[:, sl], ps[:, sl], other[:, sl])
                if mode != "partial":
                    nc.scalar.dma_start(y[tt * P:(tt + 1) * P, :], ot)

            # Pass A: arrival-matched k-extents on the two-pass tiles.
            for j in range(TSPLIT):
                do_tile(j, 0, KA[j], "partial")
            # Phase 2: interleave full-k single-pass tiles with the accum
            # passes of the two-pass tiles.
            order = []
            acc = iter(range(TSPLIT))
            for j in range(TT - TSPLIT):
                order.append(("single", TSPLIT + j))
                if j >= 1:
                    a = next(acc, None)
                    if a is not None:
                        order.append(("accum", a))
            for kind, j in order:
                if kind == "single":
                    do_tile(j, 0, KP, "single")
                else:
                    do_tile(j, KA[j], KP, "accum")

    nc.compile()
    return nc


def kernel(x, w_q, s_exp, bias):
    global last_exec_time_ns
    from concourse.bass_utils import run_bass_kernel_spmd

    x = np.asarray(x)
    w_q = np.asarray(w_q)
    s_exp = np.asarray(s_exp)
    bias = np.asarray(bias, dtype=np.float32)
    assert x.shape == (B, T, IN) and w_q.shape == (OUT, IN)

    # Fold the power-of-two per-output-channel scale into the ternary
    # weights (exact in fp8e4m3 for s in [-8, 0]) and permute channels so
    # the NCORR largest-scale ones sit in columns 0:NCORR.
    scale = np.exp2(s_exp.astype(np.float32))
    w_scaled_t = (w_q.astype(np.float32) * scale[:, None]).T  # [IN, OUT]
    order = np.argsort(-s_exp, kind="stable")
    inv_order = np.argsort(order)
    wp_f32 = np.ascontiguousarray(w_scaled_t[:, order])
    w_fp8 = wp_f32.astype(ml_dtypes.float8_e4m3fn)
    assert np.array_equal(w_fp8.astype(np.float32), wp_f32), \
        "scaled ternary weights not exact in fp8e4m3"
    assert int(s_exp[order[NCORR - 1]]) >= -3, \
        "correction channels need s >= -3 for exact wc"
    wc_f32 = wp_f32[:, :NCORR] * np.float32(2.0 ** -6)
    wc_fp8 = np.ascontiguousarray(wc_f32).astype(ml_dtypes.float8_e4m3fn)
    assert np.array_equal(wc_fp8.astype(np.float32), wc_f32), \
        "correction weights not exact in fp8e4m3"

    bias_p = bias[order]
    bias_bcast = np.ascontiguousarray(
        np.broadcast_to(bias_p.astype(np.float32), (P, OUT)))

    # Contraction-major fp8 x + scaled fp8 residual: e8 @ wc adds
    # 64*(x - x8) * w * 2^-6 = (x - x8) @ w for the corrected channels.
    xT = np.ascontiguousarray(x.transpose(0, 2, 1))          # [B, IN, T] f32
    x8 = xT.astype(ml_dtypes.float8_e4m3fn)
    e8 = ((xT - x8.astype(np.float32)) * np.float32(64.0)).astype(
        ml_dtypes.float8_e4m3fn)

    nc = _CACHE.get("nc")
    if nc is None:
        nc = _CACHE["nc"] = _build()

    in_maps = [
        {"x8": x8[b], "e8": e8[b], "w": w_fp8, "wc": wc_fp8,
         "bias": bias_bcast} for b in range(B)
    ]

    trace = bool(int(os.environ.get("BITLIN_TRACE", "0")))
    if trace:
        _install_prof_shim()
    res = run_bass_kernel_spmd(nc, in_maps, list(range(NCORES)), trace=trace)
    last_exec_time_ns = res.exec_time_ns

    out = np.stack([res.results[b]["y"] for b in range(B)], axis=0)
    # undo the output-channel permutation
    out = out[:, :, inv_order]
    return np.ascontiguousarray(out.astype(np.float32, copy=False))


# revision 8
# speedup vs baseline: 1.3714x; 1.0107x over previous
"""BitLinear (ternary weight) inference kernel for Trainium2, 8-core SPMD.

Full-input contract: kernel(**inputs) takes the complete tensors and returns
the complete output. The batch dim (B=8) is sharded 1:1 onto the 8
NeuronCores; each core computes y[b] = x[b] @ (w_q * 2^s_exp)^T + bias as a
2048^3 matmul in fp8 with DoubleRow perf mode (2 fp8 MACs/cell/cycle).

Numerics: pure-fp8 x quantization gives rel err 0.024 (> the 2e-2 gate), so
the kernel adds a correction matmul on the NCORR=512 output channels with the
largest 2^s scale (they dominate the error): host permutes the channels so
those are columns 0:512, and e = 64*(x - fp8(x)) quantized to fp8 against
wc = w * 2^-6 accumulates the correction into the same PSUM bank as the main
matmul (64 * 2^-6 = 1). Measured rel err 5.8e-3. All weight values are exact
in fp8e4m3 (+-2^s, s in [-8,0]; wc needs s >= -3, satisfied by construction).

Device schedule (PE-bound; ~137-155us/core fp8 DoubleRow floor vs 218.5us
fp16): per (row tile, k-pair) 4 main matmuls [128,2,128]x[128,2,512] plus 1
correction matmul, PSUM fp32. Two-pass tiles 0..5 with arrival-matched pass-A
k-extents park partials in SBUF so the PE never waits on the 14.25 MiB input
stream (sync ring, ~400 GB/s measured); stores ride the scalar ring.
"""
import os

import ml_dtypes
import numpy as np

B, T, IN, OUT = 8, 2048, 2048, 2048
P = 128
NCORES = 8
NF = 512        # matmul free dim (one PSUM bank of fp32)
KP = 8          # k-pairs (K=256 per DoubleRow matmul)
NCORR = 512     # corrected output channels (permuted to the front)
TSPLIT = 7
KA = [2, 3, 4, 5, 5, 5, 5]   # pass-A k-pair extent per two-pass tile
NHALF = 5       # pairs whose x/e front halves load first (pass A reads t<896)

last_exec_time_ns = None
_CACHE = {}


def _install_prof_shim():
    """Make antenv.axon_hooks importable so trace=True works under axon."""
    import sys
    import types

    if "antenv.axon_hooks" in sys.modules:
        return
    try:
        from trn_agent_boot.trn_boot import _ntff_profile_via_ctypes
    except ImportError:
        return
    hook = _ntff_profile_via_ctypes("/opt/axon/libaxon_pjrt.so")
    mod = types.ModuleType("antenv.axon_hooks")
    mod.get_axon_ntff_profile_hook = lambda: hook
    mod.set_axon_ntff_profile_hook = lambda h: None
    sys.modules["antenv.axon_hooks"] = mod


def _build():
    import concourse.bacc as bacc
    import concourse.mybir as mybir
    from concourse.tile import TileContext

    DR = mybir.MatmulPerfMode.DoubleRow

    nc = bacc.Bacc()
    x8 = nc.dram_tensor("x8", (IN, T), mybir.dt.float8e4, kind="ExternalInput")
    e8 = nc.dram_tensor("e8", (IN, T), mybir.dt.float8e4, kind="ExternalInput")
    w = nc.dram_tensor("w", (IN, OUT), mybir.dt.float8e4, kind="ExternalInput")
    wc = nc.dram_tensor("wc", (IN, NCORR), mybir.dt.float8e4,
                        kind="ExternalInput")
    bias = nc.dram_tensor("bias", (P, OUT), mybir.dt.float32, kind="ExternalInput")
    y = nc.dram_tensor("y", (T, OUT), mybir.dt.float32, kind="ExternalOutput")

    TT = T // P     # output row tiles
    OC = OUT // NF  # psum banks per row tile
    HT = T // 2

    # DoubleRow k-pair layout: [p, pair, free] where row = pair*... is
    # (2m+i)*128 + p for tile m slice [:, 2m:2m+2, :].
    x3 = x8.rearrange("(ko p) t -> p ko t", p=P)
    e3 = e8.rearrange("(ko p) t -> p ko t", p=P)
    w3 = w.rearrange("(ko p) o -> p ko o", p=P)
    wc3 = wc.rearrange("(ko p) o -> p ko o", p=P)

    with TileContext(nc) as tc:
        with tc.tile_pool(name="wp", bufs=1) as wp, \
             tc.tile_pool(name="xp", bufs=1) as xp, \
             tc.tile_pool(name="ep", bufs=1) as ep, \
             tc.tile_pool(name="wcp", bufs=1) as wcp, \
             tc.tile_pool(name="bp", bufs=1) as bp, \
             tc.tile_pool(name="op", bufs=4) as op_, \
             tc.tile_pool(name="ptp", bufs=1) as ptp, \
             tc.tile_pool(name="pp", bufs=2, space="PSUM") as pp:

            bias_t = bp.tile([P, OUT], mybir.dt.float32, tag="bias")

            # HAM pre-warm: dummy-matmul burst (~9us: ~10 cold then warm)
            # filling the initial DMA ramp, so the PE clock gate is 8/8 and
            # stays 8/8 when the first real matmuls start at ~12us.
            warm_sb = bp.tile([P, NF], mybir.dt.float16, tag="warm")
            nc.gpsimd.memset(warm_sb, 0.0)
            warm_ps = pp.tile([P, OUT], mybir.dt.float32, tag="ps",
                              name="warmps")
            NWARM = 32
            for i in range(NWARM):
                nc.tensor.matmul(warm_ps[:, :NF], warm_sb[:, :P], warm_sb,
                                 start=(i == 0), stop=(i == NWARM - 1))

            w_tiles = [None] * KP
            wc_tiles = [None] * KP
            x_tiles = [None] * KP
            e_tiles = [None] * KP

            def load_pair(m, t_hi):
                """Queue loads for k-pair m (x/e up to t_hi columns)."""
                wt = wp.tile([P, 2, OUT], mybir.dt.float8e4, tag=f"w{m}")
                nc.sync.dma_start(wt, w3[:, 2 * m:2 * m + 2, :])
                w_tiles[m] = wt
                wct = wcp.tile([P, 2, NCORR], mybir.dt.float8e4, tag=f"wc{m}")
                nc.sync.dma_start(wct, wc3[:, 2 * m:2 * m + 2, :])
                wc_tiles[m] = wct
                xt = xp.tile([P, 2, T], mybir.dt.float8e4, tag=f"x{m}")
                nc.sync.dma_start(xt[:, :, :t_hi], x3[:, 2 * m:2 * m + 2, :t_hi])
                x_tiles[m] = xt
                et = ep.tile([P, 2, T], mybir.dt.float8e4, tag=f"e{m}")
                nc.sync.dma_start(et[:, :, :t_hi], e3[:, 2 * m:2 * m + 2, :t_hi])
                e_tiles[m] = et

            # Pass-A pairs first with only the t<1024 halves of x/e (pass-A
            # tiles 0..TSPLIT-1 read t<896); the rest streams in behind and
            # the back halves (read only by single-pass tiles 8+) come last.
            for m in range(NHALF):
                load_pair(m, HT)
            nc.sync.dma_start(bias_t, bias[:, :])
            for m in range(NHALF, KP):
                load_pair(m, T)
            for m in range(NHALF):
                nc.sync.dma_start(x_tiles[m][:, :, HT:],
                                  x3[:, 2 * m:2 * m + 2, HT:])
                nc.sync.dma_start(e_tiles[m][:, :, HT:],
                                  e3[:, 2 * m:2 * m + 2, HT:])

            partial_tiles = [
                ptp.tile([P, OUT], mybir.dt.float32, tag=f"pt{j}", name=f"pt{j}")
                for j in range(TSPLIT)
            ]

            def do_tile(tt, p_lo, p_hi, mode):
                # mode: "partial" = bias add into SBUF partial (no store),
                #       "accum" = add SBUF partial + store,
                #       "single" = bias add + store
                ps = pp.tile([P, OUT], mybir.dt.float32, tag="ps",
                             name=f"ps{tt}.{p_lo}")
                for pr in range(p_lo, p_hi):
                    lhsT = x_tiles[pr][:, :, tt * P:(tt + 1) * P]
                    for oc in range(OC):
                        nc.tensor.matmul(
                            ps[:, oc * NF:(oc + 1) * NF],
                            lhsT,
                            w_tiles[pr][:, :, oc * NF:(oc + 1) * NF],
                            start=(pr == p_lo),
                            stop=(pr == p_hi - 1 and oc > 0),
                            perf_mode=DR,
                        )
                    # correction accumulates into bank 0 (channels 0:512)
                    nc.tensor.matmul(
                        ps[:, 0:NF],
                        e_tiles[pr][:, :, tt * P:(tt + 1) * P],
                        wc_tiles[pr],
                        start=False,
                        stop=(pr == p_hi - 1),
                        perf_mode=DR,
                    )
                if mode == "partial":
                    ot = partial_tiles[tt]
                else:
                    ot = op_.tile([P, OUT], mybir.dt.float32, tag="out")
                other = partial_tiles[tt] if mode == "accum" else bias_t
                if split_store and mode != "partial":
                    # tail tiles: chunk epilogue+store so stores overlap the
                    # remaining adds and split across both DMA rings
                    for q in range(OC):
                        sl = slice(q * NF, (q + 1) * NF)
                        nc.vector.tensor_add(ot[:, sl], ps[:, sl], other[:, sl])
                        eng = nc.scalar if q % 2 == 0 else nc.sync
                        eng.dma_start(y[tt * P:(tt + 1) * P, sl], ot[:, sl])
                    return
                for h in range(2):
                    sl = slice(h * OUT // 2, (h + 1) * OUT // 2)
                    nc.vector.tensor_add(ot[:, sl], ps[:, sl], other[:, sl])
                if mode != "partial":
                    store_eng.dma_start(y[tt * P:(tt + 1) * P, :], ot)

            # Pass A: arrival-matched k-extents on the two-pass tiles.
            store_eng = nc.scalar
            split_store = False
            for j in range(TSPLIT):
                do_tile(j, 0, KA[j], "partial")
            # Phase 2: interleave full-k single-pass tiles with the accum
            # passes of the two-pass tiles; stores alternate between the
            # scalar and sync rings (sync is idle once the input is in).
            order = []
            acc = iter(range(TSPLIT))
            for j in range(TT - TSPLIT):
                order.append(("single", TSPLIT + j))
                a = next(acc, None)
                if a is not None:
                    order.append(("accum", a))
            for i, (kind, j) in enumerate(order):
                store_eng = nc.scalar if i % 2 == 0 else nc.sync
                split_store = i >= len(order) - 2
                if kind == "single":
                    do_tile(j, 0, KP, "single")
                else:
                    do_tile(j, KA[j], KP, "accum")

    nc.compile()
    return nc


def kernel(x, w_q, s_exp, bias):
    global last_exec_time_ns
    from concourse.bass_utils import run_bass_kernel_spmd

    x = np.asarray(x)
    w_q = np.asarray(w_q)
    s_exp = np.asarray(s_exp)
    bias = np.asarray(bias, dtype=np.float32)
    assert x.shape == (B, T, IN) and w_q.shape == (OUT, IN)

    # Fold the power-of-two per-output-channel scale into the ternary
    # weights (exact in fp8e4m3 for s in [-8, 0]) and permute channels so
    # the NCORR largest-scale ones sit in columns 0:NCORR.
    scale = np.exp2(s_exp.astype(np.float32))
    w_scaled_t = (w_q.astype(np.float32) * scale[:, None]).T  # [IN, OUT]
    order = np.argsort(-s_exp, kind="stable")
    inv_order = np.argsort(order)
    wp_f32 = np.ascontiguousarray(w_scaled_t[:, order])
    w_fp8 = wp_f32.astype(ml_dtypes.float8_e4m3fn)
    assert np.array_equal(w_fp8.astype(np.float32), wp_f32), \
        "scaled ternary weights not exact in fp8e4m3"
    assert int(s_exp[order[NCORR - 1]]) >= -3, \
        "correction channels need s >= -3 for exact wc"
    wc_f32 = wp_f32[:, :NCORR] * np.float32(2.0 ** -6)
    wc_fp8 = np.ascontiguousarray(wc_f32).astype(ml_dtypes.float8_e4m3fn)
    assert np.array_equal(wc_fp8.astype(np.float32), wc_f32), \
        "correction weights not exact in fp8e4m3"

    bias_p = bias[order]
    bias_bcast = np.ascontiguousarray(
        np.broadcast_to(bias_p.astype(np.float32), (P, OUT)))

    # Contraction-major fp8 x + scaled fp8 residual: e8 @ wc adds
    # 64*(x - x8) * w * 2^-6 = (x - x8) @ w for the corrected channels.
    xT = np.ascontiguousarray(x.transpose(0, 2, 1))          # [B, IN, T] f32
    x8 = xT.astype(ml_dtypes.float8_e4m3fn)
    e8 = ((xT - x8.astype(np.float32)) * np.float32(64.0)).astype(
        ml_dtypes.float8_e4m3fn)

    nc = _CACHE.get("nc")
    if nc is None:
        nc = _CACHE["nc"] = _build()

    in_maps = [
        {"x8": x8[b], "e8": e8[b], "w": w_fp8, "wc": wc_fp8,
         "bias": bias_bcast} for b in range(B)
    ]

    trace = bool(int(os.environ.get("BITLIN_TRACE", "0")))
    if trace:
        _install_prof_shim()
    res = run_bass_kernel_spmd(nc, in_maps, list(range(NCORES)), trace=trace)
    last_exec_time_ns = res.exec_time_ns

    out = np.stack([res.results[b]["y"] for b in range(B)], axis=0)
    # undo the output-channel permutation
    out = out[:, :, inv_order]
    return np.ascontiguousarray(out.astype(np.float32, copy=False))


# revision 9
# speedup vs baseline: 1.5325x; 1.1174x over previous
"""BitLinear (ternary weight) inference kernel for Trainium2, 8-core SPMD.

Full-input contract: kernel(**inputs) takes the complete tensors and returns
the complete output. The batch dim (B=8) is sharded 1:1 onto the 8
NeuronCores; each core computes y[b] = x[b] @ (w_q * 2^s_exp)^T + bias as a
2048^3 matmul in fp8 with DoubleRow perf mode (2 fp8 MACs/cell/cycle).

Numerics: pure-fp8 x quantization gives rel err 0.024 (> the 2e-2 gate), so
the kernel adds a correction matmul on the NCORR=512 output channels with the
largest 2^s scale (they dominate the error): host permutes the channels so
those are columns 0:512, and e = 64*(x - fp8(x)) quantized to fp8 against
wc = w * 2^-6 accumulates the correction into the same PSUM bank as the main
matmul (64 * 2^-6 = 1). Measured rel err 5.8e-3. All weight values are exact
in fp8e4m3 (+-2^s, s in [-8,0]; wc needs s >= -3, satisfied by construction).

Device schedule (PE-bound; ~137-155us/core fp8 DoubleRow floor vs 218.5us
fp16): per (row tile, k-pair) 4 main matmuls [128,2,128]x[128,2,512] plus 1
correction matmul, PSUM fp32. Two-pass tiles 0..5 with arrival-matched pass-A
k-extents park partials in SBUF so the PE never waits on the 14.25 MiB input
stream (sync ring, ~400 GB/s measured); stores ride the scalar ring.
"""
import os

import ml_dtypes
import numpy as np

B, T, IN, OUT = 8, 2048, 2048, 2048
P = 128
NCORES = 8
NF = 512        # matmul free dim (one PSUM bank of fp32)
KP = 8          # k-pairs (K=256 per DoubleRow matmul)
NCORR = 320     # corrected output channels (permuted to the front)
TSPLIT = 7
KA = [2, 3, 4, 5, 5, 5, 5]   # pass-A k-pair extent per two-pass tile
NHALF = 5       # pairs whose x/e front halves load first (pass A reads t<896)

last_exec_time_ns = None
_CACHE = {}


def _install_prof_shim():
    """Make antenv.axon_hooks importable so trace=True works under axon."""
    import sys
    import types

    if "antenv.axon_hooks" in sys.modules:
        return
    try:
        from trn_agent_boot.trn_boot import _ntff_profile_via_ctypes
    except ImportError:
        return
    hook = _ntff_profile_via_ctypes("/opt/axon/libaxon_pjrt.so")
    mod = types.ModuleType("antenv.axon_hooks")
    mod.get_axon_ntff_profile_hook = lambda: hook
    mod.set_axon_ntff_profile_hook = lambda h: None
    sys.modules["antenv.axon_hooks"] = mod


def _build():
    import concourse.bacc as bacc
    import concourse.mybir as mybir
    from concourse.tile import TileContext

    DR = mybir.MatmulPerfMode.DoubleRow

    nc = bacc.Bacc()
    x8 = nc.dram_tensor("x8", (IN, T), mybir.dt.float8e4, kind="ExternalInput")
    e8 = nc.dram_tensor("e8", (IN, T), mybir.dt.float8e4, kind="ExternalInput")
    w = nc.dram_tensor("w", (IN, OUT), mybir.dt.float8e4, kind="ExternalInput")
    wc = nc.dram_tensor("wc", (IN, NCORR), mybir.dt.float8e4,
                        kind="ExternalInput")
    bias = nc.dram_tensor("bias", (P, OUT), mybir.dt.float32, kind="ExternalInput")
    y = nc.dram_tensor("y", (T, OUT), mybir.dt.float32, kind="ExternalOutput")

    TT = T // P     # output row tiles
    OC = OUT // NF  # psum banks per row tile
    HT = T // 2

    # DoubleRow k-pair layout: [p, pair, free] where row = pair*... is
    # (2m+i)*128 + p for tile m slice [:, 2m:2m+2, :].
    x3 = x8.rearrange("(ko p) t -> p ko t", p=P)
    e3 = e8.rearrange("(ko p) t -> p ko t", p=P)
    w3 = w.rearrange("(ko p) o -> p ko o", p=P)
    wc3 = wc.rearrange("(ko p) o -> p ko o", p=P)

    with TileContext(nc) as tc:
        with tc.tile_pool(name="wp", bufs=1) as wp, \
             tc.tile_pool(name="xp", bufs=1) as xp, \
             tc.tile_pool(name="ep", bufs=1) as ep, \
             tc.tile_pool(name="wcp", bufs=1) as wcp, \
             tc.tile_pool(name="bp", bufs=1) as bp, \
             tc.tile_pool(name="op", bufs=4) as op_, \
             tc.tile_pool(name="ptp", bufs=1) as ptp, \
             tc.tile_pool(name="pp", bufs=2, space="PSUM") as pp:

            bias_t = bp.tile([P, OUT], mybir.dt.float32, tag="bias")

            # HAM pre-warm: dummy-matmul burst (~9us: ~10 cold then warm)
            # filling the initial DMA ramp, so the PE clock gate is 8/8 and
            # stays 8/8 when the first real matmuls start at ~12us.
            warm_sb = bp.tile([P, NF], mybir.dt.float16, tag="warm")
            nc.vector.memset(warm_sb, 0.0)
            warm_ps = pp.tile([P, OUT], mybir.dt.float32, tag="ps",
                              name="warmps")
            NWARM = 36
            for i in range(NWARM):
                nc.tensor.matmul(warm_ps[:, :NF], warm_sb[:, :P], warm_sb,
                                 start=(i == 0), stop=(i == NWARM - 1))

            w_tiles = [None] * KP
            wc_tiles = [None] * KP
            x_tiles = [None] * KP
            e_tiles = [None] * KP

            def load_pair(m, t_hi):
                """Queue loads for k-pair m (x/e up to t_hi columns).
                w/x ride the sync ring; wc/e ride the scalar ring (idle
                until stores begin) so neither stream gates the other."""
                wt = wp.tile([P, 2, OUT], mybir.dt.float8e4, tag=f"w{m}")
                nc.sync.dma_start(wt, w3[:, 2 * m:2 * m + 2, :])
                w_tiles[m] = wt
                wct = wcp.tile([P, 2, NCORR], mybir.dt.float8e4, tag=f"wc{m}")
                nc.scalar.dma_start(wct, wc3[:, 2 * m:2 * m + 2, :])
                wc_tiles[m] = wct
                xt = xp.tile([P, 2, T], mybir.dt.float8e4, tag=f"x{m}")
                nc.sync.dma_start(xt[:, :, :t_hi], x3[:, 2 * m:2 * m + 2, :t_hi])
                x_tiles[m] = xt
                et = ep.tile([P, 2, T], mybir.dt.float8e4, tag=f"e{m}")
                nc.scalar.dma_start(et[:, :, :t_hi], e3[:, 2 * m:2 * m + 2, :t_hi])
                e_tiles[m] = et

            # bias first (epilogues need it), then pass-A pairs with only
            # the t<1024 halves of x/e (pass-A tiles 0..TSPLIT-1 read
            # t<896); the back halves (read only by single-pass tiles 8+)
            # come last.
            nc.scalar.dma_start(bias_t, bias[:, :])
            for m in range(NHALF):
                load_pair(m, HT)
            for m in range(NHALF, KP):
                load_pair(m, T)
            for m in range(NHALF):
                nc.sync.dma_start(x_tiles[m][:, :, HT:],
                                  x3[:, 2 * m:2 * m + 2, HT:])
                nc.scalar.dma_start(e_tiles[m][:, :, HT:],
                                    e3[:, 2 * m:2 * m + 2, HT:])

            partial_tiles = [
                ptp.tile([P, OUT], mybir.dt.float32, tag=f"pt{j}", name=f"pt{j}")
                for j in range(TSPLIT)
            ]

            def do_tile(tt, p_lo, p_hi, mode):
                # mode: "partial" = bias add into SBUF partial (no store),
                #       "accum" = add SBUF partial + store,
                #       "single" = bias add + store
                ps = pp.tile([P, OUT], mybir.dt.float32, tag="ps",
                             name=f"ps{tt}.{p_lo}")
                for pr in range(p_lo, p_hi):
                    lhsT = x_tiles[pr][:, :, tt * P:(tt + 1) * P]
                    for oc in range(OC):
                        nc.tensor.matmul(
                            ps[:, oc * NF:(oc + 1) * NF],
                            lhsT,
                            w_tiles[pr][:, :, oc * NF:(oc + 1) * NF],
                            start=(pr == p_lo),
                            stop=(pr == p_hi - 1 and oc > 0),
                            perf_mode=DR,
                        )
                    # correction accumulates into bank 0 (channels 0:NCORR)
                    nc.tensor.matmul(
                        ps[:, 0:NCORR],
                        e_tiles[pr][:, :, tt * P:(tt + 1) * P],
                        wc_tiles[pr],
                        start=False,
                        stop=(pr == p_hi - 1),
                        perf_mode=DR,
                    )
                if mode == "partial":
                    ot = partial_tiles[tt]
                else:
                    ot = op_.tile([P, OUT], mybir.dt.float32, tag="out")
                other = partial_tiles[tt] if mode == "accum" else bias_t
                if split_store and mode != "partial":
                    # tail tiles: chunk epilogue+store so stores overlap the
                    # remaining adds and split across both DMA rings
                    for q in range(OC):
                        sl = slice(q * NF, (q + 1) * NF)
                        nc.vector.tensor_add(ot[:, sl], ps[:, sl], other[:, sl])
                        eng = nc.scalar if q % 2 == 0 else nc.sync
                        eng.dma_start(y[tt * P:(tt + 1) * P, sl], ot[:, sl])
                    return
                for h in range(2):
                    sl = slice(h * OUT // 2, (h + 1) * OUT // 2)
                    nc.vector.tensor_add(ot[:, sl], ps[:, sl], other[:, sl])
                if mode != "partial":
                    store_eng.dma_start(y[tt * P:(tt + 1) * P, :], ot)

            # Pass A: arrival-matched k-extents on the two-pass tiles.
            store_eng = nc.scalar
            split_store = False
            for j in range(TSPLIT):
                do_tile(j, 0, KA[j], "partial")
            # Phase 2: interleave full-k single-pass tiles with the accum
            # passes of the two-pass tiles; stores alternate between the
            # scalar and sync rings (sync is idle once the input is in).
            order = []
            acc = iter(range(TSPLIT))
            for j in range(TT - TSPLIT):
                order.append(("single", TSPLIT + j))
                a = next(acc, None)
                if a is not None:
                    order.append(("accum", a))
            for i, (kind, j) in enumerate(order):
                store_eng = nc.scalar if i % 2 == 0 else nc.sync
                split_store = i >= len(order) - 4
                if kind == "single":
                    do_tile(j, 0, KP, "single")
                else:
                    do_tile(j, KA[j], KP, "accum")

    nc.compile()
    return nc


def kernel(x, w_q, s_exp, bias):
    global last_exec_time_ns
    from concourse.bass_utils import run_bass_kernel_spmd

    x = np.asarray(x)
    w_q = np.asarray(w_q)
    s_exp = np.asarray(s_exp)
    bias = np.asarray(bias, dtype=np.float32)
    assert x.shape == (B, T, IN) and w_q.shape == (OUT, IN)

    # Fold the power-of-two per-output-channel scale into the ternary
    # weights (exact in fp8e4m3 for s in [-8, 0]) and permute channels so
    # the NCORR largest-scale ones sit in columns 0:NCORR.
    scale = np.exp2(s_exp.astype(np.float32))
    w_scaled_t = (w_q.astype(np.float32) * scale[:, None]).T  # [IN, OUT]
    order = np.argsort(-s_exp, kind="stable")
    inv_order = np.argsort(order)
    wp_f32 = np.ascontiguousarray(w_scaled_t[:, order])
    w_fp8 = wp_f32.astype(ml_dtypes.float8_e4m3fn)
    assert np.array_equal(w_fp8.astype(np.float32), wp_f32), \
        "scaled ternary weights not exact in fp8e4m3"
    assert int(s_exp[order[NCORR - 1]]) >= -3, \
        "correction channels need s >= -3 for exact wc"
    wc_f32 = wp_f32[:, :NCORR] * np.float32(2.0 ** -6)
    wc_fp8 = np.ascontiguousarray(wc_f32).astype(ml_dtypes.float8_e4m3fn)
    assert np.array_equal(wc_fp8.astype(np.float32), wc_f32), \
        "correction weights not exact in fp8e4m3"

    bias_p = bias[order]
    bias_bcast = np.ascontiguousarray(
        np.broadcast_to(bias_p.astype(np.float32), (P, OUT)))

    # Contraction-major fp8 x + scaled fp8 residual: e8 @ wc adds
    # 64*(x - x8) * w * 2^-6 = (x - x8) @ w for the corrected channels.
    xT = np.ascontiguousarray(x.transpose(0, 2, 1))          # [B, IN, T] f32
    x8 = xT.astype(ml_dtypes.float8_e4m3fn)
    e8 = ((xT - x8.astype(np.float32)) * np.float32(64.0)).astype(
        ml_dtypes.float8_e4m3fn)

    nc = _CACHE.get("nc")
    if nc is None:
        nc = _CACHE["nc"] = _build()

    in_maps = [
        {"x8": x8[b], "e8": e8[b], "w": w_fp8, "wc": wc_fp8,
         "bias": bias_bcast} for b in range(B)
    ]

    trace = bool(int(os.environ.get("BITLIN_TRACE", "0")))
    if trace:
        _install_prof_shim()
    res = run_bass_kernel_spmd(nc, in_maps, list(range(NCORES)), trace=trace)
    last_exec_time_ns = res.exec_time_ns

    out = np.stack([res.results[b]["y"] for b in range(B)], axis=0)
    # undo the output-channel permutation
    out = out[:, :, inv_order]
    return np.ascontiguousarray(out.astype(np.float32, copy=False))


# revision 14
# speedup vs baseline: 1.5477x; 1.0099x over previous
"""BitLinear (ternary weight) inference kernel for Trainium2, 8-core SPMD.

Full-input contract: kernel(**inputs) takes the complete tensors and returns
the complete output. The batch dim (B=8) is sharded 1:1 onto the 8
NeuronCores; each core computes y[b] = x[b] @ (w_q * 2^s_exp)^T + bias as a
2048^3 matmul in fp8 with DoubleRow perf mode (2 fp8 MACs/cell/cycle).

Numerics: pure-fp8 x quantization gives rel err 0.024 (> the 2e-2 gate), so
the kernel adds a correction matmul on the NCORR=320 output channels with the
largest 2^s scale (they dominate the error; covering all s=0 channels is what
matters): host permutes the channels so those are columns 0:NCORR, and
e = 64*(x - fp8(x)) quantized to fp8 against wc = w * 2^-6 accumulates the
correction into the same PSUM bank as the main matmul (64 * 2^-6 = 1).
Measured rel err 1.15e-2 (deterministic; 1.7x under the gate). All weight
values are exact in fp8e4m3 (+-2^s, s in [-8,0]; wc needs s >= -3).

Device schedule (PE-bound; ~128us/core fp8 DoubleRow matmul floor vs 218.5us
fp16): per (row tile, k-pair) 4 main matmuls [128,2,128]x[128,2,512] plus 1
correction matmul, fp32 PSUM in two half-width (2-bank) tiles per row tile.
Two-pass tiles 0..6 with arrival-matched pass-A k-extents park partials in
SBUF so the PE never waits on the input stream (w/x on the sync ring, wc/e/
bias on the scalar ring, ~375 GB/s each); stores alternate across both rings
and the last tile drains column-half 1 early to shorten the tail.
"""
import os

import ml_dtypes
import numpy as np

B, T, IN, OUT = 8, 2048, 2048, 2048
P = 128
NCORES = 8
NF = 512        # matmul free dim (one PSUM bank of fp32)
KP = 8          # k-pairs (K=256 per DoubleRow matmul)
NCORR = 320     # corrected output channels (permuted to the front)
TSPLIT = 7
KA = [2, 3, 4, 5, 5, 5, 5]   # pass-A k-pair extent per two-pass tile
NHALF = 5       # pairs whose x/e front halves load first (pass A reads t<896)

last_exec_time_ns = None
_CACHE = {}


def _install_prof_shim():
    """Make antenv.axon_hooks importable so trace=True works under axon."""
    import sys
    import types

    if "antenv.axon_hooks" in sys.modules:
        return
    try:
        from trn_agent_boot.trn_boot import _ntff_profile_via_ctypes
    except ImportError:
        return
    hook = _ntff_profile_via_ctypes("/opt/axon/libaxon_pjrt.so")
    mod = types.ModuleType("antenv.axon_hooks")
    mod.get_axon_ntff_profile_hook = lambda: hook
    mod.set_axon_ntff_profile_hook = lambda h: None
    sys.modules["antenv.axon_hooks"] = mod


def _build():
    import concourse.bacc as bacc
    import concourse.mybir as mybir
    from concourse.tile import TileContext

    DR = mybir.MatmulPerfMode.DoubleRow

    nc = bacc.Bacc()
    x8 = nc.dram_tensor("x8", (IN, T), mybir.dt.float8e4, kind="ExternalInput")
    e8 = nc.dram_tensor("e8", (IN, T), mybir.dt.float8e4, kind="ExternalInput")
    w = nc.dram_tensor("w", (IN, OUT), mybir.dt.float8e4, kind="ExternalInput")
    wc = nc.dram_tensor("wc", (IN, NCORR), mybir.dt.float8e4,
                        kind="ExternalInput")
    bias = nc.dram_tensor("bias", (P, OUT), mybir.dt.float32, kind="ExternalInput")
    y = nc.dram_tensor("y", (T, OUT), mybir.dt.float32, kind="ExternalOutput")

    TT = T // P     # output row tiles
    OC = OUT // NF  # psum banks per row tile
    HT = T // 2
    HPS = OUT // 2  # psum tile width (2 banks); two tiles per row tile

    # DoubleRow k-pair layout: [p, pair, free] where row = pair*... is
    # (2m+i)*128 + p for tile m slice [:, 2m:2m+2, :].
    x3 = x8.rearrange("(ko p) t -> p ko t", p=P)
    e3 = e8.rearrange("(ko p) t -> p ko t", p=P)
    w3 = w.rearrange("(ko p) o -> p ko o", p=P)
    wc3 = wc.rearrange("(ko p) o -> p ko o", p=P)

    with TileContext(nc) as tc:
        with tc.tile_pool(name="wp", bufs=1) as wp, \
             tc.tile_pool(name="xp", bufs=1) as xp, \
             tc.tile_pool(name="ep", bufs=1) as ep, \
             tc.tile_pool(name="wcp", bufs=1) as wcp, \
             tc.tile_pool(name="bp", bufs=1) as bp, \
             tc.tile_pool(name="op", bufs=4) as op_, \
             tc.tile_pool(name="ptp", bufs=1) as ptp, \
             tc.tile_pool(name="pp", bufs=4, space="PSUM") as pp:

            bias_t = bp.tile([P, OUT], mybir.dt.float32, tag="bias")

            # HAM pre-warm: dummy-matmul burst (~8us: ~10 cold then warm)
            # filling the initial DMA ramp, so the PE clock gate is 8/8 and
            # stays 8/8 when the first real matmuls start at ~11us. gpsimd
            # boots in ~2.6us (the fastest memset-capable engine), so the
            # burst covers ~2.7..10.5us.
            warm_sb = bp.tile([P, NF], mybir.dt.float16, tag="warm")
            nc.gpsimd.memset(warm_sb, 0.0)
            warm_ps = pp.tile([P, HPS], mybir.dt.float32, tag="ps",
                              name="warmps")
            NWARM = 26
            for i in range(NWARM):
                nc.tensor.matmul(warm_ps[:, :NF], warm_sb[:, :P], warm_sb,
                                 start=(i == 0), stop=(i == NWARM - 1))

            w_tiles = [None] * KP
            wc_tiles = [None] * KP
            x_tiles = [None] * KP
            e_tiles = [None] * KP

            def load_pair(m, t_hi):
                """Queue loads for k-pair m (x/e up to t_hi columns).
                w/x ride the sync ring; wc/e ride the scalar ring (idle
                until stores begin) so neither stream gates the other."""
                wt = wp.tile([P, 2, OUT], mybir.dt.float8e4, tag=f"w{m}")
                nc.sync.dma_start(wt, w3[:, 2 * m:2 * m + 2, :])
                w_tiles[m] = wt
                wct = wcp.tile([P, 2, NCORR], mybir.dt.float8e4, tag=f"wc{m}")
                nc.scalar.dma_start(wct, wc3[:, 2 * m:2 * m + 2, :])
                wc_tiles[m] = wct
                xt = xp.tile([P, 2, T], mybir.dt.float8e4, tag=f"x{m}")
                nc.sync.dma_start(xt[:, :, :t_hi], x3[:, 2 * m:2 * m + 2, :t_hi])
                x_tiles[m] = xt
                et = ep.tile([P, 2, T], mybir.dt.float8e4, tag=f"e{m}")
                nc.scalar.dma_start(et[:, :, :t_hi], e3[:, 2 * m:2 * m + 2, :t_hi])
                e_tiles[m] = et

            # bias first (epilogues need it), then pass-A pairs with only
            # the t<1024 halves of x/e (pass-A tiles 0..TSPLIT-1 read
            # t<896); the back halves (read only by single-pass tiles 8+)
            # come last.
            nc.scalar.dma_start(bias_t, bias[:, :])
            for m in range(NHALF):
                load_pair(m, HT)
            for m in range(NHALF, KP):
                load_pair(m, T)
            for m in range(NHALF):
                nc.sync.dma_start(x_tiles[m][:, :, HT:],
                                  x3[:, 2 * m:2 * m + 2, HT:])
                nc.scalar.dma_start(e_tiles[m][:, :, HT:],
                                    e3[:, 2 * m:2 * m + 2, HT:])

            partial_tiles = [
                ptp.tile([P, OUT], mybir.dt.float32, tag=f"pt{j}", name=f"pt{j}")
                for j in range(TSPLIT)
            ]

            def mm_half(ps, h, tt, pr, p_lo, p_hi):
                """The two main matmuls of column half h for k-pair pr, plus
                (for h == 0) the correction matmul into channels 0:NCORR."""
                lhsT = x_tiles[pr][:, :, tt * P:(tt + 1) * P]
                for q in range(2):
                    oc = 2 * h + q
                    nc.tensor.matmul(
                        ps[:, q * NF:(q + 1) * NF],
                        lhsT,
                        w_tiles[pr][:, :, oc * NF:(oc + 1) * NF],
                        start=(pr == p_lo),
                        stop=(pr == p_hi - 1 and oc > 0),
                        perf_mode=DR,
                    )
                if h == 0:
                    nc.tensor.matmul(
                        ps[:, 0:NCORR],
                        e_tiles[pr][:, :, tt * P:(tt + 1) * P],
                        wc_tiles[pr],
                        start=False,
                        stop=(pr == p_hi - 1),
                        perf_mode=DR,
                    )

            def epilogue_half(ps, h, tt, ot, other, split):
                base = h * HPS
                if split:
                    # chunk the add+store so stores overlap the remaining
                    # adds and split across both DMA rings
                    for q in range(2):
                        sl = slice(base + q * NF, base + (q + 1) * NF)
                        nc.vector.tensor_add(ot[:, sl], ps[:, q * NF:(q + 1) * NF],
                                             other[:, sl])
                        eng = nc.scalar if (h + q) % 2 == 0 else nc.sync
                        eng.dma_start(y[tt * P:(tt + 1) * P, sl], ot[:, sl])
                else:
                    sl = slice(base, base + HPS)
                    nc.vector.tensor_add(ot[:, sl], ps, other[:, sl])

            def do_tile(tt, p_lo, p_hi, mode, last=False):
                # mode: "partial" = bias add into SBUF partial (no store),
                #       "accum" = add SBUF partial + store,
                #       "single" = bias add + store
                # Two half-width psum tiles (2 banks each) so the slots
                # recycle at half-tile granularity.
                pss = [pp.tile([P, HPS], mybir.dt.float32, tag="ps",
                               name=f"ps{tt}.{p_lo}.{h}") for h in range(2)]
                if mode == "partial":
                    ot = partial_tiles[tt]
                else:
                    ot = op_.tile([P, OUT], mybir.dt.float32, tag="out")
                other = partial_tiles[tt] if mode == "accum" else bias_t
                split = split_store and mode != "partial"
                if last:
                    # final group: run column half 1 first and drain its
                    # epilogue+stores while half 0 (incl. correction) still
                    # has matmuls in flight -> shorter post-matmul tail.
                    for pr in range(p_lo, p_hi):
                        mm_half(pss[1], 1, tt, pr, p_lo, p_hi)
                    epilogue_half(pss[1], 1, tt, ot, other, split)
                    for pr in range(p_lo, p_hi):
                        mm_half(pss[0], 0, tt, pr, p_lo, p_hi)
                    epilogue_half(pss[0], 0, tt, ot, other, split)
                else:
                    for pr in range(p_lo, p_hi):
                        for h in range(2):
                            mm_half(pss[h], h, tt, pr, p_lo, p_hi)
                    for h in range(2):
                        epilogue_half(pss[h], h, tt, ot, other, split)
                if mode != "partial" and not split:
                    store_eng.dma_start(y[tt * P:(tt + 1) * P, :], ot)

            # Pass A: arrival-matched k-extents on the two-pass tiles.
            store_eng = nc.scalar
            split_store = False
            for j in range(TSPLIT):
                do_tile(j, 0, KA[j], "partial")
            # Phase 2: interleave full-k single-pass tiles with the accum
            # passes of the two-pass tiles; stores alternate between the
            # scalar and sync rings (sync is idle once the input is in).
            order = []
            acc = iter(range(TSPLIT))
            for j in range(TT - TSPLIT):
                order.append(("single", TSPLIT + j))
                a = next(acc, None)
                if a is not None:
                    order.append(("accum", a))
            for i, (kind, j) in enumerate(order):
                store_eng = nc.scalar if i % 2 == 0 else nc.sync
                split_store = i >= len(order) - 4
                last = i == len(order) - 1
                if kind == "single":
                    do_tile(j, 0, KP, "single", last=last)
                else:
                    do_tile(j, KA[j], KP, "accum", last=last)

    nc.compile()
    return nc


def kernel(x, w_q, s_exp, bias):
    global last_exec_time_ns
    from concourse.bass_utils import run_bass_kernel_spmd

    x = np.asarray(x)
    w_q = np.asarray(w_q)
    s_exp = np.asarray(s_exp)
    bias = np.asarray(bias, dtype=np.float32)
    assert x.shape == (B, T, IN) and w_q.shape == (OUT, IN)

    # Fold the power-of-two per-output-channel scale into the ternary
    # weights (exact in fp8e4m3 for s in [-8, 0]) and permute channels so
    # the NCORR largest-scale ones sit in columns 0:NCORR.
    scale = np.exp2(s_exp.astype(np.float32))
    w_scaled_t = (w_q.astype(np.float32) * scale[:, None]).T  # [IN, OUT]
    order = np.argsort(-s_exp, kind="stable")
    inv_order = np.argsort(order)
    wp_f32 = np.ascontiguousarray(w_scaled_t[:, order])
    w_fp8 = wp_f32.astype(ml_dtypes.float8_e4m3fn)
    assert np.array_equal(w_fp8.astype(np.float32), wp_f32), \
        "scaled ternary weights not exact in fp8e4m3"
    assert int(s_exp[order[NCORR - 1]]) >= -3, \
        "correction channels need s >= -3 for exact wc"
    wc_f32 = wp_f32[:, :NCORR] * np.float32(2.0 ** -6)
    wc_fp8 = np.ascontiguousarray(wc_f32).astype(ml_dtypes.float8_e4m3fn)
    assert np.array_equal(wc_fp8.astype(np.float32), wc_f32), \
        "correction weights not exact in fp8e4m3"

    bias_p = bias[order]
    bias_bcast = np.ascontiguousarray(
        np.broadcast_to(bias_p.astype(np.float32), (P, OUT)))

    # Contraction-major fp8 x + scaled fp8 residual: e8 @ wc adds
    # 64*(x - x8) * w * 2^-6 = (x - x8) @ w for the corrected channels.
    xT = np.ascontiguousarray(x.transpose(0, 2, 1))          # [B, IN, T] f32
    x8 = xT.astype(ml_dtypes.float8_e4m3fn)
    e8 = ((xT - x8.astype(np.float32)) * np.float32(64.0)).astype(
        ml_dtypes.float8_e4m3fn)

    nc = _CACHE.get("nc")
    if nc is None:
        nc = _CACHE["nc"] = _build()

    in_maps = [
        {"x8": x8[b], "e8": e8[b], "w": w_fp8, "wc": wc_fp8,
         "bias": bias_bcast} for b in range(B)
    ]

    trace = bool(int(os.environ.get("BITLIN_TRACE", "0")))
    if trace:
        _install_prof_shim()
    res = run_bass_kernel_spmd(nc, in_maps, list(range(NCORES)), trace=trace)
    last_exec_time_ns = res.exec_time_ns

    out = np.stack([res.results[b]["y"] for b in range(B)], axis=0)
    # undo the output-channel permutation
    out = out[:, :, inv_order]
    return np.ascontiguousarray(out.astype(np.float32, copy=False))
